# revision 8
# baseline (speedup 1.0000x reference)
"""Trainium2 Bass kernel for nn_KolmogorovLayer (dense_mlp).

Math (reference):
    h   = tanh(x[:,:,None] * W1 + b1)              # [B, D, I]
    psi = einsum('bdi,dio->bdo', h, W2) + b2       # [B, D, I]
    hg  = tanh(psi.reshape(B, D*I) @ Wg1 + bg1)    # [B, I]
    out = hg @ Wg2 + bg2                           # [B, 1]

Because psi feeds a linear layer, fold W2 and Wg1 on the host:
    Meff[(d,i), f] = sum_o W2[d,i,o] * Wg1[d*I+o, f]
    beff[f]        = bg1[f] + b2.reshape(-1) @ Wg1
    u = h.reshape(B, D*I) @ Meff + beff;  out = tanh(u) @ Wg2 + bg2
This halves the matmul FLOPs. The kernel then runs a transposed pipeline
per core (batch sharded 8 ways, 4096 rows/core):
    - DMA-transpose x -> xT [64, 4096] in SBUF
    - per 128-row (d,i)-chunk: a K=2 selection matmul broadcasts the two
      needed xT rows onto 128 PSUM partitions; ScalarE computes
      h = tanh(W1*z + b1) with per-partition scale/bias (fused for free)
    - main contraction accumulates u in PSUM over 32 chunks (fp32r matmuls)
    - ScalarE tanh(u + beff), PE matvec with Wg2, +bg2 on VectorE, DMA out.
"""

import numpy as np
from contextlib import ExitStack

import concourse.bass as bass
import concourse.bacc as bacc
import concourse.mybir as mybir
import concourse.tile as tile
from concourse.bass import ts, ds
from concourse.bass_utils import run_bass_kernel_spmd
from concourse.masks import make_identity

F32 = mybir.dt.float32
F32R = mybir.dt.float32r

B_TOT, D, I = 32768, 64, 64
N_CORES = 8
BS = B_TOT // N_CORES          # 4096 rows per core
CHUNKS = D * I // 128          # 32 chunks of 128 (d,i) rows
W = 1024                       # batch window (PSUM-limited)
NW = BS // W                   # 4 windows
NS = 512                       # matmul moving-operand stream width


def _build_program():
    nc = bacc.Bacc(
        "TRN2", target_bir_lowering=False, debug=False, num_devices=N_CORES
    )
    x_d = nc.dram_tensor("x", [BS, D], F32, kind="ExternalInput").ap()
    w1_d = nc.dram_tensor("w1c", [128, CHUNKS], F32, kind="ExternalInput").ap()
    b1_d = nc.dram_tensor("b1c", [128, CHUNKS], F32, kind="ExternalInput").ap()
    me_d = nc.dram_tensor("meffc", [128, CHUNKS * I], F32R, kind="ExternalInput").ap()
    be_d = nc.dram_tensor("beff", [I, 1], F32, kind="ExternalInput").ap()
    wg_d = nc.dram_tensor("wg2", [I, 1], F32R, kind="ExternalInput").ap()
    sel_d = nc.dram_tensor("sel", [I, CHUNKS * 128], F32R, kind="ExternalInput").ap()
    bg_d = nc.dram_tensor("bg2", [1, 1], F32, kind="ExternalInput").ap()
    y_d = nc.dram_tensor("y", [BS, 1], F32, kind="ExternalOutput").ap()
    y_row = y_d.rearrange("b one -> one b")

    # x viewed so one DMA lands [128, 8, 64] per window:
    # xbm[p, t, d] = x[t*128 + p, d]
    x_bm = x_d.rearrange("(t p) d -> p t d", p=128)

    with tile.TileContext(nc) as tc, ExitStack() as ctx:
        const = ctx.enter_context(tc.tile_pool(name="const", bufs=1))
        hpool = ctx.enter_context(tc.tile_pool(name="h", bufs=3))
        hgpool = ctx.enter_context(tc.tile_pool(name="hg", bufs=2))
        opool = ctx.enter_context(tc.tile_pool(name="osb", bufs=2))
        zpool = ctx.enter_context(tc.tile_pool(name="z", bufs=2, space="PSUM"))
        ypool = ctx.enter_context(tc.tile_pool(name="yps", bufs=1, space="PSUM"))
        vpool = ctx.enter_context(tc.tile_pool(name="vps", bufs=1, space="PSUM"))
        tpool = ctx.enter_context(tc.tile_pool(name="tps", bufs=1, space="PSUM"))

        xT = const.tile([D, BS], F32R)
        xbm = const.tile([128, CHUNKS * D], F32)
        ident = const.tile([128, 128], F32)
        w1s = const.tile([128, CHUNKS], F32)
        b1s = const.tile([128, CHUNKS], F32)
        mes = const.tile([128, CHUNKS * I], F32R)
        bes = const.tile([I, 1], F32)
        wgs = const.tile([I, 1], F32R)
        sls = const.tile([I, CHUNKS * 128], F32R)
        bgs = const.tile([1, 1], F32)

        make_identity(nc, ident[:])
        nc.sync.dma_start(w1s[:], w1_d)
        nc.sync.dma_start(b1s[:], b1_d)
        nc.sync.dma_start(mes[:], me_d)
        nc.sync.dma_start(bes[:], be_d)
        nc.sync.dma_start(wgs[:], wg_d)
        nc.sync.dma_start(sls[:], sel_d)
        nc.sync.dma_start(bgs[:], bg_d)

        for w in range(NW):
            # stage x window into SBUF batch-major, then PE-transpose to xT
            nc.sync.dma_start(
                xbm[:, ts(w, 512)].rearrange("p (t d) -> p t d", d=D),
                x_bm[:, ts(w, 8), :],
            )
            for h2 in range(2):
                tp = tpool.tile([D, 512], F32)
                for j in range(4):
                    nc.tensor.transpose(
                        tp[:, ts(j, 128)],
                        xbm[:, ds(w * 512 + (4 * h2 + j) * 64, 64)],
                        ident[:],
                    )
                nc.vector.tensor_copy(
                    xT[:, ds(w * W + h2 * 512, 512)], tp[:]
                )

        for w in range(NW):
            ypsum = ypool.tile([I, W], F32)
            for c in range(CHUNKS):
                z = zpool.tile([128, W], F32)
                for s in range(W // NS):
                    nc.tensor.matmul(
                        z[:, ts(s, NS)],
                        sls[:, ts(c, 128)],
                        xT[:, ds(w * W + s * NS, NS)],
                        start=True,
                        stop=True,
                    )
                h = hpool.tile([128, W], F32R)
                nc.scalar.activation(
                    h[:],
                    z[:],
                    mybir.ActivationFunctionType.Tanh,
                    bias=b1s[:, c : c + 1],
                    scale=w1s[:, c : c + 1],
                )
                for s in range(W // NS):
                    nc.tensor.matmul(
                        ypsum[:, ts(s, NS)],
                        mes[:, ts(c, I)],
                        h[:, ts(s, NS)],
                        start=(c == 0),
                        stop=(c == CHUNKS - 1),
                    )
            hg = hgpool.tile([I, W], F32R)
            nc.scalar.activation(
                hg[:],
                ypsum[:],
                mybir.ActivationFunctionType.Tanh,
                bias=bes[:, 0:1],
            )
            osb = opool.tile([1, W], F32)
            for s in range(W // NS):
                ops = vpool.tile([1, NS], F32)
                nc.tensor.matmul(
                    ops[:],
                    wgs[:],
                    hg[:, ts(s, NS)],
                    start=True,
                    stop=True,
                )
                nc.vector.tensor_scalar_add(osb[:, ts(s, NS)], ops[:], bgs[0:1, 0:1])
            nc.sync.dma_start(y_row[:, ts(w, W)], osb[:])

    nc.compile()
    return nc


_PROGRAM_CACHE = {}


def _get_program():
    if "nc" not in _PROGRAM_CACHE:
        _PROGRAM_CACHE["nc"] = _build_program()
    return _PROGRAM_CACHE["nc"]


def _round_f32r(a):
    """Round fp32 to the nearest value representable as bf16_hi + bf16_lo."""
    import ml_dtypes
    a = np.asarray(a, np.float32)
    hi = a.astype(ml_dtypes.bfloat16).astype(np.float32)
    lo = (a - hi).astype(ml_dtypes.bfloat16).astype(np.float32)
    return hi + lo


def _prepare_weight_maps(W1, b1, W2, b2, Wg1, bg1, Wg2, bg2):
    W1 = np.asarray(W1, np.float32)
    b1 = np.asarray(b1, np.float32)
    W2 = np.asarray(W2, np.float64)
    b2 = np.asarray(b2, np.float64)
    Wg1 = np.asarray(Wg1, np.float64)
    bg1 = np.asarray(bg1, np.float64)
    Wg2 = np.asarray(Wg2, np.float32)
    bg2 = np.asarray(bg2, np.float32)

    # Fold: Meff[(d,i), f] = sum_o W2[d,i,o] Wg1[d*I+o, f]
    Wg1r = Wg1.reshape(D, I, I)
    Meff = np.einsum("dio,dof->dif", W2, Wg1r).astype(np.float32)  # [D, I, I]
    beff = (bg1 + b2.reshape(-1) @ Wg1).astype(np.float32)  # [I]

    # Chunk layouts: chunk c covers d = 2c, 2c+1; partition p = (d_rel<<6)|i
    # w1c[p, c] = W1[2c + (p>>6), p & 63]
    w1c = W1.reshape(CHUNKS, 2 * I).T.copy()  # [128, 32]
    b1c = b1.astype(np.float32).reshape(CHUNKS, 2 * I).T.copy()
    # meffc[p, c*I + f] = Meff[2c + (p>>6), p&63, f]
    meffc = (
        Meff.reshape(CHUNKS, 2 * I, I).transpose(1, 0, 2).reshape(128, CHUNKS * I)
    ).copy()
    # sel[k, c*128 + m] = 1 where k == 2c + (m>>6): routes xT row d=2c+(m>>6)
    # to chunk-partition m
    sel = np.zeros((I, CHUNKS, 128), np.float32)
    for c in range(CHUNKS):
        sel[2 * c, c, 0:I] = 1.0
        sel[2 * c + 1, c, I:128] = 1.0
    sel = sel.reshape(I, CHUNKS * 128)
    return {
        "w1c": np.ascontiguousarray(w1c, np.float32),
        "b1c": np.ascontiguousarray(b1c, np.float32),
        "meffc": _round_f32r(np.ascontiguousarray(meffc, np.float32)),
        "beff": beff.reshape(I, 1).copy(),
        "wg2": _round_f32r(Wg2.reshape(I, 1)),
        "sel": sel,
        "bg2": bg2.reshape(1, 1).astype(np.float32).copy(),
    }


def kernel(x, W1, b1, W2, b2, Wg1, bg1, Wg2, bg2, _trace=False):
    x = np.ascontiguousarray(np.asarray(x, np.float32))
    assert x.shape == (B_TOT, D)
    wmap = _prepare_weight_maps(W1, b1, W2, b2, Wg1, bg1, Wg2, bg2)
    nc = _get_program()
    in_maps = [
        {"x": np.ascontiguousarray(x[i * BS : (i + 1) * BS]), **wmap}
        for i in range(N_CORES)
    ]
    res = run_bass_kernel_spmd(nc, in_maps, list(range(N_CORES)), trace=_trace)
    y = np.concatenate([r["y"] for r in res.results], axis=0)
    if _trace:
        kernel.last_results = res
    return y.astype(np.float32)


# revision 9
# speedup vs baseline: 268.8509x; 268.8509x over previous
"""Trainium2 Bass kernel for nn_KolmogorovLayer (dense_mlp).

Math (reference):
    h   = tanh(x[:,:,None] * W1 + b1)              # [B, D, I]
    psi = einsum('bdi,dio->bdo', h, W2) + b2       # [B, D, I]
    hg  = tanh(psi.reshape(B, D*I) @ Wg1 + bg1)    # [B, I]
    out = hg @ Wg2 + bg2                           # [B, 1]

Because psi feeds a linear layer, fold W2 and Wg1 on the host:
    Meff[(d,i), f] = sum_o W2[d,i,o] * Wg1[d*I+o, f]
    beff[f]        = bg1[f] + b2.reshape(-1) @ Wg1
    u = h.reshape(B, D*I) @ Meff + beff;  out = tanh(u) @ Wg2 + bg2
This halves the matmul FLOPs. The kernel then runs a transposed pipeline
per core (batch sharded 8 ways, 4096 rows/core):
    - DMA-transpose x -> xT [64, 4096] in SBUF
    - per 128-row (d,i)-chunk: a K=2 selection matmul broadcasts the two
      needed xT rows onto 128 PSUM partitions; ScalarE computes
      h = tanh(W1*z + b1) with per-partition scale/bias (fused for free)
    - main contraction accumulates u in PSUM over 32 chunks (fp32r matmuls)
    - ScalarE tanh(u + beff), PE matvec with Wg2, +bg2 on VectorE, DMA out.
"""

import numpy as np
from contextlib import ExitStack

import concourse.bass as bass
import concourse.bacc as bacc
import concourse.mybir as mybir
import concourse.tile as tile
from concourse.bass import ts, ds
from concourse.bass_utils import run_bass_kernel_spmd
from concourse.masks import make_identity

F32 = mybir.dt.float32
F32R = mybir.dt.float32r

B_TOT, D, I = 32768, 64, 64
N_CORES = 8
BS = B_TOT // N_CORES          # 4096 rows per core
CHUNKS = D * I // 128          # 32 chunks of 128 (d,i) rows
W = 1024                       # batch window (PSUM-limited)
NW = BS // W                   # 4 windows
NS = 512                       # matmul moving-operand stream width


def _build_program(reps: int = 1):
    nc = bacc.Bacc(
        "TRN2", target_bir_lowering=False, debug=False, num_devices=N_CORES
    )
    x_d = nc.dram_tensor("x", [BS, D], F32, kind="ExternalInput").ap()
    w1_d = nc.dram_tensor("w1c", [128, CHUNKS], F32, kind="ExternalInput").ap()
    b1_d = nc.dram_tensor("b1c", [128, CHUNKS], F32, kind="ExternalInput").ap()
    me_d = nc.dram_tensor("meffc", [128, CHUNKS * I], F32R, kind="ExternalInput").ap()
    be_d = nc.dram_tensor("beff", [I, 1], F32, kind="ExternalInput").ap()
    wg_d = nc.dram_tensor("wg2", [I, 1], F32R, kind="ExternalInput").ap()
    sel_d = nc.dram_tensor("sel", [I, CHUNKS * 128], F32R, kind="ExternalInput").ap()
    bg_d = nc.dram_tensor("bg2", [1, 1], F32, kind="ExternalInput").ap()
    y_d = nc.dram_tensor("y", [BS, 1], F32, kind="ExternalOutput").ap()
    y_row = y_d.rearrange("b one -> one b")

    # x viewed so one DMA lands [128, 8, 64] per window:
    # xbm[p, t, d] = x[t*128 + p, d]
    x_bm = x_d.rearrange("(t p) d -> p t d", p=128)

    with tile.TileContext(nc) as tc, ExitStack() as ctx:
        const = ctx.enter_context(tc.tile_pool(name="const", bufs=1))
        hpool = ctx.enter_context(tc.tile_pool(name="h", bufs=3))
        hgpool = ctx.enter_context(tc.tile_pool(name="hg", bufs=2))
        opool = ctx.enter_context(tc.tile_pool(name="osb", bufs=2))
        zpool = ctx.enter_context(tc.tile_pool(name="z", bufs=2, space="PSUM"))
        ypool = ctx.enter_context(tc.tile_pool(name="yps", bufs=1, space="PSUM"))
        vpool = ctx.enter_context(tc.tile_pool(name="vps", bufs=1, space="PSUM"))
        tpool = ctx.enter_context(tc.tile_pool(name="tps", bufs=1, space="PSUM"))

        xT = const.tile([D, BS], F32R)
        xbm = const.tile([128, CHUNKS * D], F32)
        ident = const.tile([128, 128], F32)
        w1s = const.tile([128, CHUNKS], F32)
        b1s = const.tile([128, CHUNKS], F32)
        mes = const.tile([128, CHUNKS * I], F32R)
        bes = const.tile([I, 1], F32)
        wgs = const.tile([I, 1], F32R)
        sls = const.tile([I, CHUNKS * 128], F32R)
        bgs = const.tile([1, 1], F32)

        make_identity(nc, ident[:])
        nc.sync.dma_start(w1s[:], w1_d)
        nc.sync.dma_start(b1s[:], b1_d)
        nc.sync.dma_start(mes[:], me_d)
        nc.sync.dma_start(bes[:], be_d)
        nc.sync.dma_start(wgs[:], wg_d)
        nc.sync.dma_start(sls[:], sel_d)
        nc.sync.dma_start(bgs[:], bg_d)

        if reps > 1:
            loop_ctx = tc.For_i(0, reps, 1)
            loop_ctx.__enter__()
        for w in range(NW):
            # stage x window into SBUF batch-major, then PE-transpose to xT
            nc.sync.dma_start(
                xbm[:, ts(w, 512)].rearrange("p (t d) -> p t d", d=D),
                x_bm[:, ts(w, 8), :],
            )
            for h2 in range(2):
                tp = tpool.tile([D, 512], F32)
                for j in range(4):
                    nc.tensor.transpose(
                        tp[:, ts(j, 128)],
                        xbm[:, ds(w * 512 + (4 * h2 + j) * 64, 64)],
                        ident[:],
                    )
                nc.vector.tensor_copy(
                    xT[:, ds(w * W + h2 * 512, 512)], tp[:]
                )

        for w in range(NW):
            ypsum = ypool.tile([I, W], F32)
            for c in range(CHUNKS):
                z = zpool.tile([128, W], F32)
                for s in range(W // NS):
                    nc.tensor.matmul(
                        z[:, ts(s, NS)],
                        sls[:, ts(c, 128)],
                        xT[:, ds(w * W + s * NS, NS)],
                        start=True,
                        stop=True,
                    )
                h = hpool.tile([128, W], F32R)
                nc.scalar.activation(
                    h[:],
                    z[:],
                    mybir.ActivationFunctionType.Tanh,
                    bias=b1s[:, c : c + 1],
                    scale=w1s[:, c : c + 1],
                )
                for s in range(W // NS):
                    nc.tensor.matmul(
                        ypsum[:, ts(s, NS)],
                        mes[:, ts(c, I)],
                        h[:, ts(s, NS)],
                        start=(c == 0),
                        stop=(c == CHUNKS - 1),
                    )
            hg = hgpool.tile([I, W], F32R)
            nc.scalar.activation(
                hg[:],
                ypsum[:],
                mybir.ActivationFunctionType.Tanh,
                bias=bes[:, 0:1],
            )
            osb = opool.tile([1, W], F32)
            for s in range(W // NS):
                ops = vpool.tile([1, NS], F32)
                nc.tensor.matmul(
                    ops[:],
                    wgs[:],
                    hg[:, ts(s, NS)],
                    start=True,
                    stop=True,
                )
                nc.vector.tensor_scalar_add(osb[:, ts(s, NS)], ops[:], bgs[0:1, 0:1])
            nc.sync.dma_start(y_row[:, ts(w, W)], osb[:])
        if reps > 1:
            loop_ctx.__exit__(None, None, None)

    nc.compile()
    return nc


_PROGRAM_CACHE = {}


def _get_program(reps: int = 1):
    if reps not in _PROGRAM_CACHE:
        _PROGRAM_CACHE[reps] = _build_program(reps)
    return _PROGRAM_CACHE[reps]


def _round_f32r(a):
    """Round fp32 to the nearest value representable as bf16_hi + bf16_lo."""
    import ml_dtypes
    a = np.asarray(a, np.float32)
    hi = a.astype(ml_dtypes.bfloat16).astype(np.float32)
    lo = (a - hi).astype(ml_dtypes.bfloat16).astype(np.float32)
    return hi + lo


def _prepare_weight_maps(W1, b1, W2, b2, Wg1, bg1, Wg2, bg2):
    W1 = np.asarray(W1, np.float32)
    b1 = np.asarray(b1, np.float32)
    W2 = np.asarray(W2, np.float64)
    b2 = np.asarray(b2, np.float64)
    Wg1 = np.asarray(Wg1, np.float64)
    bg1 = np.asarray(bg1, np.float64)
    Wg2 = np.asarray(Wg2, np.float32)
    bg2 = np.asarray(bg2, np.float32)

    # Fold: Meff[(d,i), f] = sum_o W2[d,i,o] Wg1[d*I+o, f]
    Wg1r = Wg1.reshape(D, I, I)
    Meff = np.einsum("dio,dof->dif", W2, Wg1r).astype(np.float32)  # [D, I, I]
    beff = (bg1 + b2.reshape(-1) @ Wg1).astype(np.float32)  # [I]

    # Chunk layouts: chunk c covers d = 2c, 2c+1; partition p = (d_rel<<6)|i
    # w1c[p, c] = W1[2c + (p>>6), p & 63]
    w1c = W1.reshape(CHUNKS, 2 * I).T.copy()  # [128, 32]
    b1c = b1.astype(np.float32).reshape(CHUNKS, 2 * I).T.copy()
    # meffc[p, c*I + f] = Meff[2c + (p>>6), p&63, f]
    meffc = (
        Meff.reshape(CHUNKS, 2 * I, I).transpose(1, 0, 2).reshape(128, CHUNKS * I)
    ).copy()
    # sel[k, c*128 + m] = 1 where k == 2c + (m>>6): routes xT row d=2c+(m>>6)
    # to chunk-partition m
    sel = np.zeros((I, CHUNKS, 128), np.float32)
    for c in range(CHUNKS):
        sel[2 * c, c, 0:I] = 1.0
        sel[2 * c + 1, c, I:128] = 1.0
    sel = sel.reshape(I, CHUNKS * 128)
    return {
        "w1c": np.ascontiguousarray(w1c, np.float32),
        "b1c": np.ascontiguousarray(b1c, np.float32),
        "meffc": _round_f32r(np.ascontiguousarray(meffc, np.float32)),
        "beff": beff.reshape(I, 1).copy(),
        "wg2": _round_f32r(Wg2.reshape(I, 1)),
        "sel": sel,
        "bg2": bg2.reshape(1, 1).astype(np.float32).copy(),
    }


def kernel(x, W1, b1, W2, b2, Wg1, bg1, Wg2, bg2, _trace=False):
    x = np.ascontiguousarray(np.asarray(x, np.float32))
    assert x.shape == (B_TOT, D)
    wmap = _prepare_weight_maps(W1, b1, W2, b2, Wg1, bg1, Wg2, bg2)
    nc = _get_program()
    in_maps = [
        {"x": np.ascontiguousarray(x[i * BS : (i + 1) * BS]), **wmap}
        for i in range(N_CORES)
    ]
    res = run_bass_kernel_spmd(nc, in_maps, list(range(N_CORES)), trace=_trace)
    y = np.concatenate([r["y"] for r in res.results], axis=0)
    if _trace:
        kernel.last_results = res
    return y.astype(np.float32)
